# revision 2
# baseline (speedup 1.0000x reference)
"""Causal attention on 8 TRN2 cores — fp8 DoubleRow scores variant (v3).

2 cores per batch; the PAIR splits the KEYS: parity c owns key blocks
S_c = sorted({2p+c} u {31-2p-c}) (16 of 32 blocks), projects K^T/V for
its 2048 keys + Q^T for all 4096 queries, computes partial unnormalized
attention, host combines y = (num0+num1)/(ell0+ell1).

v3 on top of v2 (transposed scores, QK-fold into K'):
 - scores matmul in fp8e4m3 with perf_mode=DoubleRow (2 contraction rows
   per PE cell): 4 MMs per key block instead of 8. K' is computed bf16
   (M pre-scaled x32 on host so fp8 stays in normal range) and evacuated
   PSUM->SBUF directly as fp8; queries ship from host as fp8. The exp
   activation folds the 1/32 back into its scale.
 - value chain in fp16 (V, pT, y-numerator): same PE speed as bf16, 8x
   finer mantissa; ranges validated (max num ~162, ell ~2203, p ~6).

Program structure is parity-independent (one SPMD NEFF); parity enters
only through data: which keys are in xT, and the 2 diagonal-band masks
per chunk (keyset blocks 2c and 2c+1) that cut beyond-causal entries.
"""

import math
import sys

sys.path.insert(0, "/opt/trn_rl_repo")

import ml_dtypes
import numpy as np

import concourse.mybir as mybir
import concourse.tile as tile
from concourse import bacc
from concourse.bass_utils import run_bass_kernel_spmd

B = 4
S = 4096
D = 1024
P = 128
DC = D // P          # 8 chunks of the contraction dim
NQB = 32             # query blocks per batch
NCH = 8              # query chunks (4 blocks = 512 q each)
HALF = S // 2        # keys owned per core
BF16 = mybir.dt.bfloat16
F16 = mybir.dt.float16
F8 = mybir.dt.float8e4
F32 = mybir.dt.float32
DR = mybir.MatmulPerfMode.DoubleRow
NEG = -1.0e9
SCALE = 1.0 / math.sqrt(D)
MFOLD = 32.0         # host premultiplies M by this; exp scale divides it out


def _keyset(c):
    return sorted({2 * p + c for p in range(8)} | {31 - 2 * p - c for p in range(8)})


def _L(g):
    return (g + 2) // 2  # ceil((g+1)/2): unified per-parity kv-block count


def _build_program(reps=1):
    nc = bacc.Bacc("TRN2", target_bir_lowering=False, debug=False)

    xT = nc.dram_tensor("xT", [D, HALF], BF16, kind="ExternalInput").ap()
    xTq = nc.dram_tensor("xTq", [D, S], F8, kind="ExternalInput").ap()
    # mT = 32*(Wq @ Wk.T).T — the QK bilinear form folded into one matrix, so
    # only keys get projected (K' = x_k @ M.T) and raw x serves as queries
    mT = nc.dram_tensor("mT", [D, D], BF16, kind="ExternalInput").ap()
    wv = nc.dram_tensor("wv", [D, D], BF16, kind="ExternalInput").ap()
    mask = nc.dram_tensor("mask", [2 * NCH, P, 512], F32, kind="ExternalInput").ap()
    y = nc.dram_tensor("y", [S, D], F16, kind="ExternalOutput").ap()
    ell = nc.dram_tensor("ell", [1, S], F32, kind="ExternalOutput").ap()

    with tile.TileContext(nc) as tc:
        with (
            tc.tile_pool(name="big", bufs=1) as big,
            tc.tile_pool(name="wpool", bufs=2) as wpool,
            tc.tile_pool(name="xslab", bufs=2) as xslab,
            tc.tile_pool(name="qt", bufs=2) as qt_pool,
            tc.tile_pool(name="mk", bufs=2) as mk_pool,
            tc.tile_pool(name="pT", bufs=2) as pT_pool,
            tc.tile_pool(name="yy", bufs=2) as y_pool,
            tc.tile_pool(name="els", bufs=2) as els_pool,
            tc.tile_pool(name="ps", bufs=3, space="PSUM") as ps,
            tc.tile_pool(name="yp", bufs=4, space="PSUM") as yp_pool,
            tc.tile_pool(name="elp", bufs=1, space="PSUM") as el_pool,
        ):
          for _rep in range(reps):
            KT = big.tile([P, DC, HALF], F8, tag="KT")       # 16 KB/part
            V = big.tile([P, 16, D], F16, tag="V")           # 32 KB/part
            ones = big.tile([P, 1], F16, tag="ones")
            nc.gpsimd.memset(ones[:], 1.0)

            # ---- fused K'^T + V projection over the local key half -----
            # K'^T[a, key] = sum_b M[a, b] xT[b, key]: identical matmul
            # structure to a plain K projection, with mT in place of Wk.
            wk_t = wpool.tile([P, DC, D], BF16, tag="W")
            for _i in range(DC):
                nc.scalar.dma_start(
                    out=wk_t[:, _i, :], in_=mT[_i * P : (_i + 1) * P, :]
                )
            wv_t = wpool.tile([P, DC, D], BF16, tag="W")
            for _i in range(DC):
                nc.scalar.dma_start(
                    out=wv_t[:, _i, :], in_=wv[_i * P : (_i + 1) * P, :]
                )
            for kt in range(4):  # local key tiles of 512
                xs = xslab.tile([P, DC, 512], BF16, tag="xs")
                for _i in range(DC):
                    nc.sync.dma_start(
                        out=xs[:, _i, :],
                        in_=xT[_i * P : (_i + 1) * P,
                               kt * 512 : (kt + 1) * 512],
                    )
                for j in range(DC):
                    pt = ps.tile([P, 512], F32, tag="ps", name=f"kp{kt}_{j}")
                    for i in range(DC):
                        nc.tensor.matmul(
                            pt[:],
                            lhsT=wk_t[:, i, j * P : (j + 1) * P],
                            rhs=xs[:, i, :],
                            start=(i == 0),
                            stop=(i == DC - 1),
                        )
                    nc.vector.tensor_copy(
                        KT[:, j, kt * 512 : (kt + 1) * 512], pt[:]
                    )
                for sb in range(4):
                    kb = kt * 4 + sb
                    pv = [ps.tile([P, 512], F32, tag="ps", name=f"v{n}_{kb}")
                          for n in range(2)]
                    for i in range(DC):
                        for n in range(2):
                            nc.tensor.matmul(
                                pv[n][:],
                                lhsT=xs[:, i, sb * P : (sb + 1) * P],
                                rhs=wv_t[:, i, n * 512 : (n + 1) * 512],
                                start=(i == 0),
                                stop=(i == DC - 1),
                            )
                    for n in range(2):
                        nc.scalar.copy(V[:, kb, n * 512 : (n + 1) * 512], pv[n][:])

            # ---- per query chunk: raw x as queries, scores, attn@V -----
            for c in range(NCH):
                Lmax = 2 * c + 2

                # queries are raw x columns (the W_q W_k^T fold lives in K')
                QT = qt_pool.tile([P, DC, 512], F8, tag="QT")
                for _i in range(DC):
                    nc.sync.dma_start(
                        out=QT[:, _i, :],
                        in_=xTq[_i * P : (_i + 1) * P,
                                c * 512 : (c + 1) * 512],
                    )

                # masks for the two diagonal-band key blocks (kbi 2c, 2c+1)
                mks = []
                for sl in range(2):
                    mk = mk_pool.tile([P, 512], F32, tag="mk",
                                      name=f"mk{c}_{sl}")
                    nc.scalar.dma_start(out=mk[:], in_=mask[2 * c + sl])
                    mks.append(mk)

                # transposed scores S_T[k, q] + exp -> P_T, per local kv blk.
                # fp8 DoubleRow: 2 d-chunks per MM, 4 MMs per block.
                # ell[q] = sum_k P_T[k, q] accumulates via a ones-stationary
                # matmul per block, lagged one block behind the scores so the
                # PE never waits on the exp that produces its rhs.
                pT = pT_pool.tile([P, 16, 512], F16, tag="pT")
                elps = el_pool.tile([1, 512], F32, tag="elp", name=f"elp{c}")

                def ell_mm(kbi):
                    nc.tensor.matmul(
                        elps[0:1, :],
                        lhsT=ones[:, 0:1],
                        rhs=pT[:, kbi, :],
                        start=(kbi == 0),
                        stop=(kbi == Lmax - 1),
                    )

                for kbi in range(Lmax):
                    pts = ps.tile([P, 512], F32, tag="ps", name=f"sc{c}_{kbi}")
                    for i in range(DC // 2):
                        nc.tensor.matmul(
                            pts[:],
                            lhsT=KT[:, 2 * i : 2 * i + 2,
                                    kbi * P : (kbi + 1) * P],
                            rhs=QT[:, 2 * i : 2 * i + 2, :],
                            start=(i == 0),
                            stop=(i == DC // 2 - 1),
                            perf_mode=DR,
                        )
                    if kbi >= 2 * c:
                        nc.vector.tensor_add(pts[:], pts[:], mks[kbi - 2 * c][:])
                    nc.scalar.activation(
                        pT[:, kbi, :],
                        pts[:],
                        mybir.ActivationFunctionType.Exp,
                        bias=0.0,
                        scale=SCALE / MFOLD,
                    )
                    if kbi > 0:
                        ell_mm(kbi - 1)

                # attn @ V, per query block of the chunk
                for gi in range(4):
                    g = 4 * c + gi
                    Lg = _L(g)
                    yps = [yp_pool.tile([P, 512], F32, tag="yp",
                                        name=f"y{n}_{g}")
                           for n in range(2)]
                    for kbi in range(Lg):
                        pslab = pT[:, kbi, gi * P : (gi + 1) * P]
                        for n in range(2):
                            nc.tensor.matmul(
                                yps[n][:],
                                lhsT=pslab,
                                rhs=V[:, kbi, n * 512 : (n + 1) * 512],
                                start=(kbi == 0),
                                stop=(kbi == Lg - 1),
                            )
                    if gi == 0:
                        ell_mm(Lmax - 1)
                        els = els_pool.tile([1, 512], F32, tag="els")
                        nc.vector.tensor_copy(els[0:1, :], elps[0:1, :])
                        nc.gpsimd.dma_start(
                            out=ell[0:1, c * 512 : (c + 1) * 512], in_=els[0:1, :]
                        )
                    # evacuate numerator: n=0 on scalar, n=1 on vector so
                    # neither engine bottlenecks the early (small-L) chunks
                    ys0 = y_pool.tile([P, 512], F16, tag="y")
                    nc.scalar.copy(ys0[:], yps[0][:])
                    nc.gpsimd.dma_start(
                        out=y[g * P : (g + 1) * P, 0:512], in_=ys0[:]
                    )
                    ys1 = y_pool.tile([P, 512], F16, tag="y")
                    nc.vector.tensor_copy(ys1[:], yps[1][:])
                    nc.gpsimd.dma_start(
                        out=y[g * P : (g + 1) * P, 512:1024], in_=ys1[:]
                    )
    nc.finalize()
    return nc


_NC = None


def _get_program():
    global _NC
    if _NC is None:
        _NC = _build_program()
    return _NC


def _build_mask(c):
    """mask[2*ch+sl, p, qcol]: additive mask for key block ks[2*ch+sl]
    against query chunk ch (global q = ch*512 + qcol, k = b*128 + p)."""
    ks = _keyset(c)
    m = np.zeros((2 * NCH, P, 512), np.float32)
    q = np.arange(512)[None, :]
    p = np.arange(P)[:, None]
    for ch in range(NCH):
        for sl in range(2):
            b = ks[2 * ch + sl]
            keep = (b * P + p) <= (ch * 512 + q)
            m[2 * ch + sl] = np.where(keep, 0.0, NEG)
    return m


def _make_in_maps(x, Wq, Wk, Wv, cores=range(8)):
    bf = ml_dtypes.bfloat16
    f8 = ml_dtypes.float8_e4m3
    # fold the QK bilinear form on the host: scores = x_q (Wq Wk^T) x_k^T.
    # pre-scale by MFOLD so K' lands in fp8e4m3's comfortable range.
    m = np.asarray(Wq, np.float32) @ np.asarray(Wk, np.float32).T
    mTb = np.ascontiguousarray((MFOLD * m).T.astype(bf))
    wvb = np.ascontiguousarray(Wv.astype(bf))
    masks = [_build_mask(0), _build_mask(1)]
    keycols = [
        np.concatenate([np.arange(b * P, (b + 1) * P) for b in _keyset(c)])
        for c in (0, 1)
    ]

    in_maps = []
    for core in cores:
        b, c = core // 2, core % 2
        xb = x[b]
        in_maps.append(
            {
                "xT": np.ascontiguousarray(xb[keycols[c]].T.astype(bf)),
                "xTq": np.ascontiguousarray(xb.T.astype(f8)),
                "mT": mTb,
                "wv": wvb,
                "mask": masks[c],
            }
        )
    return in_maps


def kernel(x, Wq, Wk, Wv):
    nc = _get_program()
    in_maps = _make_in_maps(x, Wq, Wk, Wv)

    res = run_bass_kernel_spmd(nc, in_maps, core_ids=list(range(8))).results

    out = np.empty((B, S, D), np.float32)
    for b in range(B):
        r0, r1 = res[2 * b], res[2 * b + 1]
        num = r0["y"].astype(np.float32) + r1["y"].astype(np.float32)
        l0 = r0["ell"].reshape(S, 1)
        l1 = r1["ell"].reshape(S, 1)
        out[b] = num / (l0 + l1)
    return out


# revision 18
# speedup vs baseline: 1.2735x; 1.2735x over previous
"""Causal attention on 8 TRN2 cores — fp8 DoubleRow scores variant (v4).

2 cores per batch; the PAIR splits the KEYS: parity c owns key blocks
S_c = sorted({2p+c} u {31-2p-c}) (16 of 32 blocks), projects K^T/V for
its 2048 keys + Q^T for all 4096 queries, computes partial unnormalized
attention, host combines y = (num0+num1)/(ell0+ell1).

v4 on top of v3 (fp8 DoubleRow scores, fp16 value chain):
 - chunk software pipelining: scores(c+1) is emitted before attn(c), so
   the PE streams scores matmuls while ACT exps the previous chunk and
   never stalls on the exp -> attn stationary handoff.
 - ell off the PE: per-block DVE accumulate (f32) + one f16 ones-matmul
   per chunk replaces the per-block N=512 ones-matmuls (72 -> 8 MMs).
 - startup: xs(kt0) DMA is emitted first; the wk weight DMA is split in
   column halves (j<4 first) so the first K' matmul group waits ~2us
   instead of ~4; wv rides the vector queue in parallel.
 - y-evac DMAs alternate gpsimd/sync queues to shorten the end drain.
"""

import math
import sys

sys.path.insert(0, "/opt/trn_rl_repo")

import ml_dtypes
import numpy as np

import concourse.mybir as mybir
import concourse.tile as tile
from concourse import bacc
from concourse.bass_utils import run_bass_kernel_spmd

B = 4
S = 4096
D = 1024
P = 128
DC = D // P          # 8 chunks of the contraction dim
NQB = 32             # query blocks per batch
NCH = 8              # query chunks (4 blocks = 512 q each)
HALF = S // 2        # keys owned per core
BF16 = mybir.dt.bfloat16
F16 = mybir.dt.float16
F8 = mybir.dt.float8e4
F32 = mybir.dt.float32
DR = mybir.MatmulPerfMode.DoubleRow
NEG = -1.0e9
SCALE = 1.0 / math.sqrt(D)
MFOLD = 32.0         # host premultiplies M by this; exp scale divides it out


def _keyset(c):
    return sorted({2 * p + c for p in range(8)} | {31 - 2 * p - c for p in range(8)})


def _L(g):
    return (g + 2) // 2  # ceil((g+1)/2): unified per-parity kv-block count


def _build_program(reps=1):
    nc = bacc.Bacc("TRN2", target_bir_lowering=False, debug=False)

    xT = nc.dram_tensor("xT", [D, HALF], BF16, kind="ExternalInput").ap()
    xTk8 = nc.dram_tensor("xTk8", [D, HALF], F8, kind="ExternalInput").ap()
    xTq = nc.dram_tensor("xTq", [D, S], F8, kind="ExternalInput").ap()
    # mT = 32*(Wq @ Wk.T).T — the QK bilinear form folded into one matrix, so
    # only keys get projected (K' = x_k @ M.T) and raw x serves as queries
    mT = nc.dram_tensor("mT", [D, D], F8, kind="ExternalInput").ap()
    wv = nc.dram_tensor("wv", [D, D], BF16, kind="ExternalInput").ap()
    mask = nc.dram_tensor("mask", [NCH, P, 1024], BF16, kind="ExternalInput").ap()
    y = nc.dram_tensor("y", [S, D], F16, kind="ExternalOutput").ap()
    ell = nc.dram_tensor("ell", [1, S], F32, kind="ExternalOutput").ap()

    with tile.TileContext(nc) as tc:
        with (
            tc.tile_pool(name="big", bufs=1) as big,
            tc.tile_pool(name="wpool", bufs=2) as wpool,
            tc.tile_pool(name="xslab", bufs=2) as xslab,
            tc.tile_pool(name="qt", bufs=2) as qt_pool,
            tc.tile_pool(name="mk", bufs=4) as mk_pool,
            tc.tile_pool(name="pT", bufs=2) as pT_pool,
            tc.tile_pool(name="ea", bufs=2) as ea_pool,
            tc.tile_pool(name="yy", bufs=2) as y_pool,
            tc.tile_pool(name="els", bufs=2) as els_pool,
            tc.tile_pool(name="ps", bufs=3, space="PSUM") as ps,
            tc.tile_pool(name="yp", bufs=4, space="PSUM") as yp_pool,
            tc.tile_pool(name="elp", bufs=1, space="PSUM") as el_pool,
        ):
          for _rep in range(reps):
            KT = big.tile([P, DC, HALF], F8, tag="KT")       # 16 KB/part
            V = big.tile([P, 16, D], F16, tag="V")           # 32 KB/part
            ones = big.tile([P, 1], F16, tag="ones")
            nc.gpsimd.memset(ones[:], 1.0)

            # ---- fused K'^T + V projection over the local key half -----
            # K'^T[a, key] = sum_b M[a, b] xT[b, key]: identical matmul
            # structure to a plain K projection, with mT in place of Wk.
            # K' runs fp8 DoubleRow (4 MMs per group); V projection stays
            # bf16. The fp8 key slabs (xs8, all 4 kt) land first on the SP
            # queue, weights in column halves on the ACT queue, wv on the
            # gpsimd queue, bf16 slabs after xs8 — ordered so no matmul
            # group waits long. K'/V interleave kt-shifted (V one kt behind)
            # because V's inputs (wv + bf16 xs) arrive later.
            xs8s = [xslab.tile([P, DC, 512], F8, tag="xs8",
                               name=f"xs8_{kt}", bufs=4) for kt in range(4)]
            xss = [xslab.tile([P, DC, 512], BF16, tag="xs",
                              name=f"xs_{kt}", bufs=4) for kt in range(4)]
            xTk8_r = xTk8.rearrange("(i p) k -> p i k", p=P)
            xT_r = xT.rearrange("(i p) k -> p i k", p=P)
            xTq_r = xTq.rearrange("(i p) k -> p i k", p=P)
            mT_r = mT.rearrange("(i p) c -> p i c", p=P)
            wv_r = wv.rearrange("(i p) c -> p i c", p=P)

            # single big DMAs (one HWDGE slot each): SP queue carries the
            # fp8 then bf16 key slabs; ACT queue carries wk halves + wv
            # quarters; gpsimd stays free for outputs.
            for kt in range(4):
                nc.sync.dma_start(
                    out=xs8s[kt][:, :, :],
                    in_=xTk8_r[:, :, kt * 512 : (kt + 1) * 512],
                )
            wk_t = wpool.tile([P, DC, D], F8, tag="Wk", bufs=1)
            for half in range(2):
                nc.scalar.dma_start(
                    out=wk_t[:, :, half * 512 : (half + 1) * 512],
                    in_=mT_r[:, :, half * 512 : (half + 1) * 512],
                )
            wv_t = wpool.tile([P, DC, D], BF16, tag="Wv", bufs=1)
            for q in range(4):
                nc.scalar.dma_start(
                    out=wv_t[:, :, q * 256 : (q + 1) * 256],
                    in_=wv_r[:, :, q * 256 : (q + 1) * 256],
                )
            for kt in range(4):
                nc.sync.dma_start(
                    out=xss[kt][:, :, :],
                    in_=xT_r[:, :, kt * 512 : (kt + 1) * 512],
                )

            def emit_kproj(kt):
                xs8 = xs8s[kt]
                for j in range(DC):
                    pt = ps.tile([P, 512], F32, tag="ps", name=f"kp{kt}_{j}")
                    for i in range(DC // 2):
                        nc.tensor.matmul(
                            pt[:],
                            lhsT=wk_t[:, 2 * i : 2 * i + 2,
                                      j * P : (j + 1) * P],
                            rhs=xs8[:, 2 * i : 2 * i + 2, :],
                            start=(i == 0),
                            stop=(i == DC // 2 - 1),
                            perf_mode=DR,
                        )
                    nc.vector.tensor_copy(
                        KT[:, j, kt * 512 : (kt + 1) * 512], pt[:]
                    )

            def emit_vproj(kt):
                xs = xss[kt]
                for sb in range(4):
                    kb = kt * 4 + sb
                    pv = [ps.tile([P, 512], F32, tag="ps", name=f"v{n}_{kb}")
                          for n in range(2)]
                    for i in range(DC):
                        for n in range(2):
                            nc.tensor.matmul(
                                pv[n][:],
                                lhsT=xs[:, i, sb * P : (sb + 1) * P],
                                rhs=wv_t[:, i, n * 512 : (n + 1) * 512],
                                start=(i == 0),
                                stop=(i == DC - 1),
                            )
                    for n in range(2):
                        nc.scalar.copy(V[:, kb, n * 512 : (n + 1) * 512], pv[n][:])

            emit_kproj(0)
            emit_kproj(1)
            emit_vproj(0)
            emit_kproj(2)
            emit_vproj(1)
            emit_kproj(3)
            emit_vproj(2)
            emit_vproj(3)

            # ---- per query chunk: raw x as queries, scores, attn@V -----
            # software pipelined: scores(c+1) is emitted before attn(c).

            def emit_scores(c):
                Lmax = 2 * c + 2

                # queries are raw x columns (the W_q W_k^T fold lives in K')
                QT = qt_pool.tile([P, DC, 512], F8, tag="QT",
                                  name=f"QT{c}")
                nc.sync.dma_start(
                    out=QT[:, :, :],
                    in_=xTq_r[:, :, c * 512 : (c + 1) * 512],
                )

                # masks for the two diagonal-band key blocks (kbi 2c, 2c+1)
                mk = mk_pool.tile([P, 1024], BF16, tag="mk", name=f"mk{c}")
                nc.scalar.dma_start(out=mk[:], in_=mask[c])
                mks = [mk[:, 0:512], mk[:, 512:1024]]

                # transposed scores S_T[k, q] + exp -> P_T, per local kv blk.
                # fp8 DoubleRow: 2 d-chunks per MM, 4 MMs per block.
                # ell accumulates on the DVE (f32), one f16 copy at the end
                # feeds the single per-chunk ones-matmul in emit_attn.
                pT = pT_pool.tile([P, 16, 512], F16, tag="pT", name=f"pT{c}")
                ell_acc = ea_pool.tile([P, 512], F32, tag="ea",
                                       name=f"ea{c}")
                for kbi in range(Lmax):
                    pts = ps.tile([P, 512], F32, tag="ps", name=f"sc{c}_{kbi}")
                    for i in range(DC // 2):
                        nc.tensor.matmul(
                            pts[:],
                            lhsT=KT[:, 2 * i : 2 * i + 2,
                                    kbi * P : (kbi + 1) * P],
                            rhs=QT[:, 2 * i : 2 * i + 2, :],
                            start=(i == 0),
                            stop=(i == DC // 2 - 1),
                            perf_mode=DR,
                        )
                    if kbi >= 2 * c:
                        nc.vector.tensor_add(pts[:], pts[:], mks[kbi - 2 * c])
                    nc.scalar.activation(
                        pT[:, kbi, :],
                        pts[:],
                        mybir.ActivationFunctionType.Exp,
                        bias=0.0,
                        scale=SCALE / MFOLD,
                    )
                    if kbi == 0:
                        nc.vector.tensor_copy(ell_acc[:], pT[:, 0, :])
                    else:
                        nc.vector.tensor_add(ell_acc[:], ell_acc[:],
                                             pT[:, kbi, :])
                ell16 = ea_pool.tile([P, 512], F16, tag="e16", name=f"e16{c}")
                nc.vector.tensor_copy(ell16[:], ell_acc[:])
                return pT, ell16

            def emit_attn(c, pT, ell16):
                # ell partition-reduce: one N=512 f16 ones-matmul per chunk
                elps = el_pool.tile([1, 512], F32, tag="elp", name=f"elp{c}")
                nc.tensor.matmul(
                    elps[0:1, :],
                    lhsT=ones[:, 0:1],
                    rhs=ell16[:],
                    start=True,
                    stop=True,
                )
                els = els_pool.tile([1, 512], F32, tag="els", name=f"els{c}")
                nc.vector.tensor_copy(els[0:1, :], elps[0:1, :])
                nc.gpsimd.dma_start(
                    out=ell[0:1, c * 512 : (c + 1) * 512], in_=els[0:1, :]
                )

                # attn @ V, per query block of the chunk
                for gi in range(4):
                    g = 4 * c + gi
                    Lg = _L(g)
                    yps = [yp_pool.tile([P, 512], F32, tag="yp",
                                        name=f"y{n}_{g}")
                           for n in range(2)]
                    for kbi in range(Lg):
                        pslab = pT[:, kbi, gi * P : (gi + 1) * P]
                        for n in range(2):
                            nc.tensor.matmul(
                                yps[n][:],
                                lhsT=pslab,
                                rhs=V[:, kbi, n * 512 : (n + 1) * 512],
                                start=(kbi == 0),
                                stop=(kbi == Lg - 1),
                            )
                    # evacuate numerator: n=0 on scalar, n=1 on vector so
                    # neither engine bottlenecks; one merged DMA per block
                    ys = y_pool.tile([P, 1024], F16, tag="y", bufs=4,
                                     name=f"ys_{g}")
                    nc.scalar.copy(ys[:, 0:512], yps[0][:])
                    nc.vector.tensor_copy(ys[:, 512:1024], yps[1][:])
                    nc.gpsimd.dma_start(
                        out=y[g * P : (g + 1) * P, :], in_=ys[:]
                    )

            prev = None
            for c in range(NCH):
                cur = (c, *emit_scores(c))
                if prev is not None:
                    emit_attn(*prev)
                prev = cur
            emit_attn(*prev)
    nc.finalize()
    return nc


_NC = None


def _get_program():
    global _NC
    if _NC is None:
        _NC = _build_program()
    return _NC


def _build_mask(c):
    """mask[ch, p, sl*512+qcol]: additive mask for key block ks[2*ch+sl]
    against query chunk ch (global q = ch*512 + qcol, k = b*128 + p)."""
    ks = _keyset(c)
    m = np.zeros((NCH, P, 1024), ml_dtypes.bfloat16)
    q = np.arange(512)[None, :]
    p = np.arange(P)[:, None]
    for ch in range(NCH):
        for sl in range(2):
            b = ks[2 * ch + sl]
            keep = (b * P + p) <= (ch * 512 + q)
            m[ch, :, sl * 512 : (sl + 1) * 512] = np.where(
                keep, 0.0, NEG
            ).astype(ml_dtypes.bfloat16)
    return m


def _make_in_maps(x, Wq, Wk, Wv, cores=range(8)):
    bf = ml_dtypes.bfloat16
    f8 = ml_dtypes.float8_e4m3
    # fold the QK bilinear form on the host: scores = x_q (Wq Wk^T) x_k^T.
    # pre-scale by MFOLD so K' lands in fp8e4m3's comfortable range.
    m = np.asarray(Wq, np.float32) @ np.asarray(Wk, np.float32).T
    mTb = np.ascontiguousarray((MFOLD * m).T.astype(f8))
    wvb = np.ascontiguousarray(Wv.astype(bf))
    masks = [_build_mask(0), _build_mask(1)]
    keycols = [
        np.concatenate([np.arange(b * P, (b + 1) * P) for b in _keyset(c)])
        for c in (0, 1)
    ]

    in_maps = []
    for core in cores:
        b, c = core // 2, core % 2
        xb = x[b]
        in_maps.append(
            {
                "xT": np.ascontiguousarray(xb[keycols[c]].T.astype(bf)),
                "xTk8": np.ascontiguousarray(xb[keycols[c]].T.astype(f8)),
                "xTq": np.ascontiguousarray(xb.T.astype(f8)),
                "mT": mTb,
                "wv": wvb,
                "mask": masks[c],
            }
        )
    return in_maps


def kernel(x, Wq, Wk, Wv):
    nc = _get_program()
    in_maps = _make_in_maps(x, Wq, Wk, Wv)

    res = run_bass_kernel_spmd(nc, in_maps, core_ids=list(range(8))).results

    out = np.empty((B, S, D), np.float32)
    for b in range(B):
        r0, r1 = res[2 * b], res[2 * b + 1]
        num = r0["y"].astype(np.float32) + r1["y"].astype(np.float32)
        l0 = r0["ell"].reshape(S, 1)
        l1 = r1["ell"].reshape(S, 1)
        out[b] = num / (l0 + l1)
    return out
